# revision 1
# baseline (speedup 1.0000x reference)
"""GCNN message-passing kernel for Trainium2 (8 NeuronCores, batch-parallel).

Reference computation per graph:
    ax  = segment_sum(vals[:, None] * x[cols], rows, N)   # sparse A @ x
    out = relu(ax @ W + b)

Sharding: one graph per NeuronCore (data parallel over batch, W/b replicated).

Per-core device strategy:
  Host prep (index marshaling only, no FLOPs): stable-bucket the edges by
  destination node tile (128 nodes per tile), pad every bucket to a fixed
  chunk count CPB (chunks of 128 edges), and build device layouts:
    - colsw : int16 gather indices, dma_gather wrapped layout
    - trowsw: fp16 destination row offset within tile, edge e at [e%128, e//128]
    - tvalsw: fp16 edge values, same layout
  Device:
    - dma_gather pulls x[cols[e]] rows (fp16, 256 B each) HBM -> SBUF; edge e
      lands on partition e%128, chunk slot e//128.
    - Per 128-edge chunk, ONE DVE tensor_scalar builds the scaled one-hot
      R'[e, m] = vals[e] * (iota[m] == trow[e])  (fp16 out, fp32 scalars).
    - TensorE accumulates psum[c, m] += G_chunk.T @ R'_chunk over the bucket's
      chunks => axT tile = (A @ X)^T[:, tile] entirely in PSUM.
    - Phase 2 per tile: psum2 = axT_i.T @ W + ones.T @ b, DVE relu, DMA out.
"""

import numpy as np
from contextlib import ExitStack

import concourse.bass as bass
import concourse.bacc as bacc
import concourse.mybir as mybir
import concourse.tile as tile
from concourse import library_config
from concourse.bass_utils import run_bass_kernel_spmd

B, N, E, C = 8, 10000, 320000, 128

F16 = mybir.dt.float16
F32 = mybir.dt.float32
I16 = mybir.dt.int16


# ---------------------------------------------------------------- host prep

def prep_graph(rows, cols, vals, nt, cpb):
    """Bucket one graph's edges by destination tile, pad, build device layouts.

    Returns (colsw [128, EP/16] i16, trowsw [128, EP/128] f16,
             tvalsw [128, EP/128] f16) where EP = nt*cpb*128.
    """
    ep = nt * cpb * 128
    e = rows.shape[0]
    bucket = (rows.astype(np.int64) >> 7)
    order = np.argsort(bucket, kind="stable")
    sb = bucket[order]
    counts = np.bincount(bucket, minlength=nt)
    starts = np.zeros(nt + 1, np.int64)
    np.cumsum(counts, out=starts[1:])
    wbi = np.arange(e, dtype=np.int64) - starts[sb]
    pos = sb * (cpb * 128) + wbi

    cols_p = np.zeros(ep, np.int16)
    vals_p = np.zeros(ep, np.float32)
    trow_p = np.zeros(ep, np.float32)
    cols_p[pos] = cols[order].astype(np.int16)
    vals_p[pos] = vals[order].astype(np.float32)
    trow_p[pos] = (rows[order].astype(np.int64) - sb * 128).astype(np.float32)

    colsw = np.tile(np.ascontiguousarray(cols_p.reshape(-1, 16).T), (8, 1))
    trowsw = np.ascontiguousarray(trow_p.reshape(-1, 128).T)
    tvalsw = np.ascontiguousarray(vals_p.reshape(-1, 128).T)
    return colsw, trowsw, tvalsw


def max_bucket_chunks(all_rows, nt):
    """CPB = max over graphs/buckets of ceil(bucket_size/128)."""
    mx = 0
    for rows in all_rows:
        counts = np.bincount(rows.astype(np.int64) >> 7, minlength=nt)
        mx = max(mx, int(counts.max()))
    return (mx + 127) // 128


# ---------------------------------------------------------------- device code

def build_nc(n_nodes, nt, cpb, num_devices=8, reps=1, n_queues=4):
    """Build the per-core bass program (same NEFF for all cores).

    reps > 1 repeats the whole compute (timing amortization only).
    n_queues: SWDGE queues; dma_gather desc-gen runs on Q7 core pair
    (2q, 2q+1), so round-robin queue_num parallelizes desc-gen 4x.
    """
    chunks = nt * cpb
    ep = chunks * 128
    nc = bacc.Bacc(
        "TRN2",
        target_bir_lowering=False,
        debug=False,
        num_devices=num_devices,
        num_swdge_queues=n_queues,
    )

    x16_d = nc.dram_tensor("x16", [n_nodes, C], F16, kind="ExternalInput")
    colsw_d = nc.dram_tensor("colsw", [128, ep // 16], I16, kind="ExternalInput")
    trows_d = nc.dram_tensor("trows", [128, chunks], F32, kind="ExternalInput")
    tvals_d = nc.dram_tensor("tvals", [128, chunks], F32, kind="ExternalInput")
    iota_d = nc.dram_tensor("iota", [128, 130], F16, kind="ExternalInput")
    w_d = nc.dram_tensor("w", [C, C], F32, kind="ExternalInput")
    b_d = nc.dram_tensor("b", [1, C], F32, kind="ExternalInput")
    out_d = nc.dram_tensor("out", [nt * 128, C], F32, kind="ExternalOutput")

    with tile.TileContext(nc) as tc, ExitStack() as ctx:
        nc.gpsimd.load_library(library_config.mlp)
        const = ctx.enter_context(tc.tile_pool(name="const", bufs=1))
        gpool = ctx.enter_context(tc.tile_pool(name="g", bufs=2))
        rpool = ctx.enter_context(tc.tile_pool(name="r", bufs=12))
        ps1 = ctx.enter_context(tc.tile_pool(name="ps1", bufs=4, space="PSUM"))
        ps2 = ctx.enter_context(tc.tile_pool(name="ps2", bufs=2, space="PSUM"))
        opool = ctx.enter_context(tc.tile_pool(name="o", bufs=4))

        colsw = const.tile([128, ep // 16], I16, tag="colsw")
        nc.sync.dma_start(colsw[:], colsw_d[:, :])
        trows = const.tile([128, chunks], F32, tag="trows")
        nc.sync.dma_start(trows[:], trows_d[:, :])
        tvals = const.tile([128, chunks], F32, tag="tvals")
        nc.sync.dma_start(tvals[:], tvals_d[:, :])
        iota = const.tile([128, 130], F16, tag="iota")
        nc.sync.dma_start(iota[:], iota_d[:, :])
        wsb = const.tile([C, C], F32, tag="w")
        nc.sync.dma_start(wsb[:], w_d[:, :])
        bsb = const.tile([1, C], F32, tag="b")
        nc.sync.dma_start(bsb[:], b_d[:, :])
        ones = const.tile([1, 128], F32, tag="ones")
        nc.vector.memset(ones[:], 1.0)
        axT = const.tile([128, nt * 128], F32, tag="axT")

        NG = n_queues  # buckets per gather group (one per SWDGE queue)
        n_groups = (nt + NG - 1) // NG
        for _rep in range(reps):
          for grp in range(n_groups):
              # All of a group's gathers share one double-buffered group tile,
              # so their slot-WAR wait clears atomically and the 4 gathers
              # dispatch back-to-back -> desc-gen runs on all 4 Q7 core pairs.
              gb = gpool.tile([128, NG * cpb, C], F16, tag="g")
              for q in range(NG):
                  i = grp * NG + q
                  if i >= nt:
                      continue
                  nc.gpsimd.dma_gather(
                      gb[:, q * cpb:(q + 1) * cpb, :],
                      x16_d[:, :],
                      colsw[:, i * cpb * 8:(i + 1) * cpb * 8],
                      num_idxs=cpb * 128,
                      num_idxs_reg=cpb * 128,
                      elem_size=C,
                      single_packet=False,
                      queue_num=q,
                  )
              for q in range(NG):
                  i = grp * NG + q
                  if i >= nt:
                      continue
                  ps = ps1.tile([C, 128], F32, tag="ps1")
                  for k in range(cpb):
                      j = i * cpb + k
                      r = rpool.tile([128, 132], F16, tag="r")
                      nc.vector.tensor_scalar(
                          r[:, 0:128], iota[:, 0:128],
                          trows[:, j:j + 1], tvals[:, j:j + 1],
                          op0=mybir.AluOpType.is_equal, op1=mybir.AluOpType.mult,
                      )
                      nc.tensor.matmul(
                          ps[:], gb[:, q * cpb + k, :], r[:, 0:128],
                          start=(k == 0), stop=(k == cpb - 1),
                      )
                  axT_i = axT[:, i * 128:(i + 1) * 128]
                  nc.scalar.copy(axT_i, ps[:])
          for i in range(nt):
              axT_i = axT[:, i * 128:(i + 1) * 128]
              ps2t = ps2.tile([128, C], F32, tag="ps2")
              nc.tensor.matmul(ps2t[:], axT_i, wsb[:], start=True, stop=False)
              nc.tensor.matmul(ps2t[:], ones[:], bsb[:], start=False, stop=True)
              ot = opool.tile([128, C], F32, tag="o")
              nc.vector.tensor_scalar(
                  ot[:], ps2t[:], 0.0, None, op0=mybir.AluOpType.max,
              )
              nc.sync.dma_start(out_d[i * 128:(i + 1) * 128, :], ot[:])

    nc.compile()
    return nc


# ---------------------------------------------------------------- entry point

_cache = {}


def _get_nc(n_nodes, nt, cpb):
    key = (n_nodes, nt, cpb)
    if key not in _cache:
        _cache[key] = build_nc(n_nodes, nt, cpb)
    return _cache[key]


def make_in_maps(x, rows, cols, vals, W, b, nt, cpb):
    nb = x.shape[0]
    iota_np = np.tile(np.arange(130, dtype=np.float16), (128, 1))
    in_maps = []
    for g in range(nb):
        colsw, trowsw, tvalsw = prep_graph(rows[g], cols[g], vals[g], nt, cpb)
        in_maps.append({
            "x16": np.ascontiguousarray(x[g].astype(np.float16)),
            "colsw": colsw,
            "trows": trowsw,
            "tvals": tvalsw,
            "iota": iota_np,
            "w": np.ascontiguousarray(W.astype(np.float32)),
            "b": np.ascontiguousarray(b.astype(np.float32)[None, :]),
        })
    return in_maps


def kernel(x, rows, cols, vals, W, b, _trace=False):
    x = np.asarray(x)
    rows = np.asarray(rows)
    cols = np.asarray(cols)
    vals = np.asarray(vals)
    W = np.asarray(W)
    b = np.asarray(b)
    nb, n_nodes, _ = x.shape
    nt = (n_nodes + 127) // 128
    cpb = max_bucket_chunks([rows[g] for g in range(nb)], nt)

    nc = _get_nc(n_nodes, nt, cpb)
    in_maps = make_in_maps(x, rows, cols, vals, W, b, nt, cpb)
    res = run_bass_kernel_spmd(
        nc, in_maps, core_ids=list(range(nb)), trace=_trace,
    )
    out = np.stack([r["out"][:n_nodes] for r in res.results]).astype(np.float32)
    if _trace:
        return out, res
    return out

